# revision 7
# baseline (speedup 1.0000x reference)
"""Trainium2 Bass kernel for batched masked Kabsch-RMSD (Coords2RMSD loss).

Problem: for each of 4096 samples (1024 max atoms, variable num_atoms),
compute RMSD after optimal rigid alignment (Kabsch). Data-parallel over
8 NeuronCores (512 samples each), samples on SBUF partitions.

Math (per sample, avoids explicit centering):
  mask_i = i < n;  xm = mask*x, ym = mask*y   (interleaved [1024,3] coords)
  Sx_j = sum_i xm_ij, Sy likewise; sxx = sum xm^2, syy = sum ym^2
  R_jk = sum_i xm_ij ym_ik
  Rc = R - Sx Sy^T / n;  ex = sxx - |Sx|^2/n;  ey = syy - |Sy|^2/n
  M = Rc^T Rc;  eigenvalues via Smith's closed form (acos/cos through
  the ScalarE Arctan/Sin tables);  d = sign(det Rc)
  s = sqrt(l1)+sqrt(l2)+d*sqrt(l3);  rmsd = sqrt(max((ex+ey-2s)/n, 1e-12))

Engine split per 128-sample tile:
  DVE : mask compare, 6 fused mask-apply+centroid-sum STTs (strided fp32
        reads, dense de-interleaved writes), 5 fused product+accum STTs
  GPS : 4 fused product+accum STTs (dense reads)
  ACT : 2 Square+accum passes for the norms
Covariance products use scalar_tensor_tensor's accum_out so no separate
reduction passes exist anywhere.
"""

import math
import numpy as np

import concourse.bass as bass
import concourse.mybir as mybir
from concourse.bass_utils import run_bass_kernel_spmd
from concourse.tile import TileContext

F32 = mybir.dt.float32
BF16 = mybir.dt.bfloat16
FP16 = mybir.dt.float16
I32 = mybir.dt.int32
ALU = mybir.AluOpType
ACT = mybir.ActivationFunctionType

N_CORES = 8
B_FULL = 4096
B_CORE = B_FULL // N_CORES        # 512
N_ATOMS = 1024
ROW = 3 * N_ATOMS                 # 3072
N_TILES = B_CORE // 128           # 4

# products assigned to gpsimd (dense reads only; rest go to DVE)
GPS_PRODUCTS = 6
GPS_MASKS = 2
BF16_PRODUCTS = True


def _build_kernel(split_waits: bool = True) -> bass.Bass:
    nc = bass.Bass()

    inp = nc.declare_dram_parameter("input", [B_CORE, ROW], F32, isOutput=False)
    tgt = nc.declare_dram_parameter("target", [B_CORE, ROW], F32, isOutput=False)
    nat = nc.declare_dram_parameter("num_atoms", [B_CORE], I32, isOutput=False)
    out = nc.declare_dram_parameter("out", [B_CORE], F32, isOutput=True)

    with TileContext(nc) as tc:
        with tc.tile_pool(name="p", bufs=1) as pool, \
             tc.tile_pool(name="io", bufs=2) as iop:

            # ---------- one-time setup ----------
            iota_i = pool.tile([128, N_ATOMS], I32, tag="iota_i")
            nc.gpsimd.iota(iota_i[:], pattern=[[1, N_ATOMS]], base=0,
                           channel_multiplier=0)
            # fp16 iota: integers <= 2048 exact; enables DVE fast-mode is_lt
            iota_h = pool.tile([128, N_ATOMS], FP16, tag="iota_h")
            nc.vector.tensor_copy(iota_h[:], iota_i[:])

            # num_atoms -> [128, N_TILES] i32 -> f32, inv_n
            n_i = pool.tile([128, N_TILES], I32, tag="n_i")
            nc.sync.dma_start(out=n_i[:],
                              in_=nat[:].rearrange("(t p) -> p t", p=128))
            nf = pool.tile([128, N_TILES], F32, tag="nf")
            nc.vector.tensor_copy(nf[:], n_i[:])
            inv_n = pool.tile([128, N_TILES], F32, tag="inv_n")
            nc.vector.reciprocal(inv_n[:], nf[:])

            # ---------- per-core stats ----------
            # stats[:, t, 0:9] = R9, [:, t, 9:12] = Sx, [:, t, 12:15] = Sy
            stats = pool.tile([128, N_TILES, 15], F32, tag="stats")
            R9 = stats[:, :, 0:9]
            Sx = stats[:, :, 9:12]
            Sy = stats[:, :, 12:15]
            sxx = pool.tile([128, N_TILES], F32, tag="sxx")
            syy = pool.tile([128, N_TILES], F32, tag="syy")

            act_scr = pool.tile([128, ROW], BF16, tag="act_scr")

            # ---------- main loop over 4 sample tiles ----------
            # Engine split (measured rates): DVE TT bf16 dense hits the 2x
            # fast mode (~0.68us/[128,1024]); STT has no fast mode (~2.4us)
            # so it is banned here. GPS shares DVE's SBUF read port and
            # poisons it (TT 0.68 -> 2.6us when GPS runs) so GPS is idle.
            # ACT (own port) does the strided de-interleaves + squares.
            for t in range(N_TILES):
                x = iop.tile([128, ROW], F32, tag="x")
                y = iop.tile([128, ROW], F32, tag="y")
                nc.sync.dma_start(out=x[:], in_=inp[t * 128 : (t + 1) * 128, :])
                nc.sync.dma_start(out=y[:], in_=tgt[t * 128 : (t + 1) * 128, :])

                # ACT: de-interleave + fp32->bf16 (strided read, dense write)
                xd = iop.tile([128, 3, N_ATOMS], BF16, tag="xd")
                yd = iop.tile([128, 3, N_ATOMS], BF16, tag="yd")
                nc.scalar.activation(xd[:], x[:].rearrange("p (n c) -> p c n", c=3),
                                     ACT.Copy)
                nc.scalar.activation(yd[:], y[:].rearrange("p (n c) -> p c n", c=3),
                                     ACT.Copy)

                # DVE: mask (fast-mode TS), mask-apply (fast-mode TT w/ bcast)
                mask = iop.tile([128, N_ATOMS], BF16, tag="mask")
                nc.vector.tensor_scalar(mask[:], iota_h[:], nf[:, t : t + 1],
                                        None, ALU.is_lt)
                mb = mask[:].rearrange("p n -> p () n").broadcast_to(
                    (128, 3, N_ATOMS))

                # red[:, 0:9] products, [:, 9:12] xm, [:, 12:15] ym
                red = iop.tile([128, 15, N_ATOMS], BF16, tag="red")
                xm = red[:, 9:12, :]
                ym = red[:, 12:15, :]
                nc.vector.tensor_tensor(xm, xd[:], mb, ALU.mult)
                nc.vector.tensor_tensor(ym, yd[:], mb, ALU.mult)

                # 9 covariance products in one fast-mode TT: outer-product
                # broadcast APs (j over xm, k over ym), dense bf16 out
                xmb = red[:, 9:12, :].rearrange("p j n -> p j () n") \
                    .broadcast_to((128, 3, 3, N_ATOMS))
                ymb = red[:, 12:15, :].rearrange("p k n -> p () k n") \
                    .broadcast_to((128, 3, 3, N_ATOMS))
                nc.vector.tensor_tensor(
                    red[:, 0:9, :].rearrange("p (j k) n -> p j k n", k=3),
                    xmb, ymb, ALU.mult)

                # reduce [128, 15, 1024] -> [128, 15] fp32. A direct
                # tensor_reduce runs 1x (fp32 out kills the 2-byte fast
                # mode; measured 16.1us). Binary-fold in fp16 (2x TT) down
                # to 64, then one small 1x reduce (~9.4us total).
                f1 = iop.tile([128, 15, 512], FP16, tag="f1")
                nc.vector.tensor_tensor(f1[:], red[:, :, 0:512],
                                        red[:, :, 512:1024], ALU.add)
                f2 = iop.tile([128, 15, 256], FP16, tag="f2")
                nc.vector.tensor_tensor(f2[:], f1[:, :, 0:256],
                                        f1[:, :, 256:512], ALU.add)
                f3 = iop.tile([128, 15, 128], FP16, tag="f3")
                nc.vector.tensor_tensor(f3[:], f2[:, :, 0:128],
                                        f2[:, :, 128:256], ALU.add)
                f4 = iop.tile([128, 15, 64], FP16, tag="f4")
                nc.vector.tensor_tensor(f4[:], f3[:, :, 0:64],
                                        f3[:, :, 64:128], ALU.add)
                nc.vector.tensor_reduce(stats[:, t, :], f4[:],
                                        mybir.AxisListType.X, ALU.add)

                # norms on ScalarE (Square + accumulate over the whole row)
                nc.scalar.activation(act_scr[:],
                                     red[:, 9:12, :].rearrange("p c n -> p (c n)"),
                                     ACT.Square, accum_out=sxx[:, t : t + 1])
                nc.scalar.activation(act_scr[:],
                                     red[:, 12:15, :].rearrange("p c n -> p (c n)"),
                                     ACT.Square, accum_out=syy[:, t : t + 1])

            # ---------- batched eigensolve / RMSD ([128, N_TILES]) ----------
            T = N_TILES

            def tile4(shape, tag):
                return pool.tile(shape, F32, tag=tag, name=tag)

            v = nc.vector
            s_ = nc.scalar


            # Rc = R9 - (Sx outer Sy) * inv_n
            Rc = tile4([128, T, 9], "Rc")
            t9a = tile4([128, T, 9], "t9a")
            sxb = Sx.broadcast_to((128, T, 3, 3))
            syb = Sy.rearrange("p t k -> p t () k").broadcast_to((128, T, 3, 3))
            v.tensor_tensor(t9a[:].rearrange("p t (j k) -> p t j k", k=3),
                            sxb, syb, ALU.mult)
            invb9 = inv_n[:].rearrange("p t -> p t ()").broadcast_to((128, T, 9))
            v.tensor_tensor(t9a[:], t9a[:], invb9, ALU.mult)
            v.tensor_tensor(Rc[:], R9, t9a[:], ALU.subtract)

            # ex = sxx - |Sx|^2 * inv_n ; ey likewise
            t3 = tile4([128, T, 3], "t3")
            tA = tile4([128, T], "tA")
            tB = tile4([128, T], "tB")
            ex = tile4([128, T], "ex")
            ey = tile4([128, T], "ey")
            v.tensor_tensor(t3[:], Sx, Sx, ALU.mult)
            v.tensor_reduce(tA[:], t3[:], mybir.AxisListType.X, ALU.add)
            v.tensor_tensor(tA[:], tA[:], inv_n[:], ALU.mult)
            v.tensor_tensor(ex[:], sxx[:], tA[:], ALU.subtract)
            v.tensor_tensor(t3[:], Sy, Sy, ALU.mult)
            v.tensor_reduce(tB[:], t3[:], mybir.AxisListType.X, ALU.add)
            v.tensor_tensor(tB[:], tB[:], inv_n[:], ALU.mult)
            v.tensor_tensor(ey[:], syy[:], tB[:], ALU.subtract)

            def col(ap, i):            # [128, T] column i of a [128,T,9] tile
                return ap[:, :, i]

            r00, r01, r02 = (col(Rc, i) for i in range(3))
            r10, r11, r12 = (col(Rc, i) for i in range(3, 6))
            r20, r21, r22 = (col(Rc, i) for i in range(6, 9))

            # det(Rc) via cofactors
            c0 = tile4([128, T], "c0")
            c1 = tile4([128, T], "c1")
            c2 = tile4([128, T], "c2")
            u0 = tile4([128, T], "u0")
            u1 = tile4([128, T], "u1")
            det = tile4([128, T], "det")

            def msub(dst, a, b, c, d):     # dst = a*b - c*d
                v.tensor_tensor(u0[:], a, b, ALU.mult)
                v.tensor_tensor(u1[:], c, d, ALU.mult)
                v.tensor_tensor(dst, u0[:], u1[:], ALU.subtract)

            g = nc.gpsimd
            gu0 = tile4([128, T], "gu0")
            gu1 = tile4([128, T], "gu1")

            def gmsub(dst, a, b, c, d):    # dst = a*b - c*d  (on GPS)
                g.tensor_tensor(gu0[:], a, b, ALU.mult)
                g.tensor_tensor(gu1[:], c, d, ALU.mult)
                g.tensor_tensor(dst, gu0[:], gu1[:], ALU.subtract)

            gc0 = tile4([128, T], "gc0")
            gc1 = tile4([128, T], "gc1")
            gc2 = tile4([128, T], "gc2")
            gmsub(gc0[:], r11, r22, r12, r21)
            gmsub(gc1[:], r10, r22, r12, r20)
            gmsub(gc2[:], r10, r21, r11, r20)
            g.tensor_tensor(gc0[:], gc0[:], r00, ALU.mult)
            g.tensor_tensor(gc1[:], gc1[:], r01, ALU.mult)
            g.tensor_tensor(gc2[:], gc2[:], r02, ALU.mult)
            g.tensor_tensor(det[:], gc0[:], gc1[:], ALU.subtract)
            g.tensor_tensor(det[:], det[:], gc2[:], ALU.add)

            # M = Rc^T Rc (6 unique entries)
            M6 = tile4([128, T, 6], "M6")      # M00 M11 M22 M01 M02 M12
            Rcv = Rc[:].rearrange("p t (j k) -> p t j k", k=3)
            mpairs = [(0, 0), (1, 1), (2, 2), (0, 1), (0, 2), (1, 2)]
            for i, (a, b) in enumerate(mpairs):
                v.tensor_tensor(t3[:], Rcv[:, :, :, a], Rcv[:, :, :, b], ALU.mult)
                v.tensor_reduce(M6[:, :, i], t3[:], mybir.AxisListType.X, ALU.add)

            M00, M11, M22 = (M6[:, :, i] for i in range(3))
            M01, M02, M12 = (M6[:, :, i] for i in range(3, 6))

            # q = tr/3 ; p = sqrt(p2/6) with p2 = sum aii^2 + 2*(off^2)
            q = tile4([128, T], "q")
            v.tensor_tensor(q[:], M00, M11, ALU.add)
            v.tensor_tensor(q[:], q[:], M22, ALU.add)
            v.tensor_scalar(q[:], q[:], 1.0 / 3.0, None, ALU.mult)

            a00 = tile4([128, T], "a00")
            a11 = tile4([128, T], "a11")
            a22 = tile4([128, T], "a22")
            v.tensor_tensor(a00[:], M00, q[:], ALU.subtract)
            v.tensor_tensor(a11[:], M11, q[:], ALU.subtract)
            v.tensor_tensor(a22[:], M22, q[:], ALU.subtract)

            p2 = tile4([128, T], "p2")
            v.tensor_tensor(u0[:], M01, M01, ALU.mult)
            v.tensor_tensor(u1[:], M02, M02, ALU.mult)
            v.tensor_tensor(p2[:], u0[:], u1[:], ALU.add)
            v.tensor_tensor(u0[:], M12, M12, ALU.mult)
            v.tensor_tensor(p2[:], p2[:], u0[:], ALU.add)
            v.tensor_scalar(p2[:], p2[:], 2.0, None, ALU.mult)
            v.tensor_tensor(u0[:], a00[:], a00[:], ALU.mult)
            v.tensor_tensor(p2[:], p2[:], u0[:], ALU.add)
            v.tensor_tensor(u0[:], a11[:], a11[:], ALU.mult)
            v.tensor_tensor(p2[:], p2[:], u0[:], ALU.add)
            v.tensor_tensor(u0[:], a22[:], a22[:], ALU.mult)
            v.tensor_tensor(p2[:], p2[:], u0[:], ALU.add)

            pp = tile4([128, T], "pp")
            v.tensor_scalar(pp[:], p2[:], 1.0 / 6.0, None, ALU.mult)
            s_.activation(pp[:], pp[:], ACT.Sqrt)
            ip = tile4([128, T], "ip")
            v.tensor_scalar(u0[:], pp[:], 1e-20, None, ALU.max)
            v.reciprocal(ip[:], u0[:])

            # detB = det(M - qI) ; r = detB * ip^3 / 2, clamped to [-1, 1]
            detB = tile4([128, T], "detB")
            msub(c0[:], a11[:], a22[:], M12, M12)
            msub(c1[:], M01, a22[:], M12, M02)
            msub(c2[:], M01, M12, a11[:], M02)
            v.tensor_tensor(c0[:], c0[:], a00[:], ALU.mult)
            v.tensor_tensor(c1[:], c1[:], M01, ALU.mult)
            v.tensor_tensor(c2[:], c2[:], M02, ALU.mult)
            v.tensor_tensor(detB[:], c0[:], c1[:], ALU.subtract)
            v.tensor_tensor(detB[:], detB[:], c2[:], ALU.add)

            rr = tile4([128, T], "rr")
            v.tensor_tensor(u0[:], ip[:], ip[:], ALU.mult)
            v.tensor_tensor(u0[:], u0[:], ip[:], ALU.mult)
            v.tensor_tensor(rr[:], detB[:], u0[:], ALU.mult)
            v.tensor_scalar(rr[:], rr[:], 0.5, None, ALU.mult)
            v.tensor_scalar(rr[:], rr[:], 1.0, -1.0, ALU.min, ALU.max)

            # c = cos(acos(r)/3) is the root of 4c^3-3c=r in [1/2,1].
            # Newton (table-free, avoids ACT Arctan/Sin set loads):
            #   c <- (8c^3 + r) / (12c^2 - 3), from c=1, 7 iterations.
            # Errors at the degenerate r=-1 corner are benign: lam1+lam2 is
            # trace-compensated and d(t3)/dc = 0 at c=1/2.
            cc = tile4([128, T], "cc")
            u2 = tile4([128, T], "u2")
            # init: quadratic fit of cos(acos(r)/3), max err ~5e-3
            v.tensor_scalar(cc[:], rr[:], -0.116, 0.25, ALU.mult, ALU.add)
            v.scalar_tensor_tensor(cc[:], rr[:], 1.0, cc[:], ALU.mult, ALU.mult)
            v.tensor_scalar(cc[:], cc[:], 1.0, 0.866, ALU.mult, ALU.add)
            for _ in range(3):
                v.tensor_tensor(u0[:], cc[:], cc[:], ALU.mult)          # c^2
                v.tensor_scalar(u2[:], u0[:], 12.0, -3.0, ALU.mult, ALU.add)
                v.tensor_tensor(u0[:], u0[:], cc[:], ALU.mult)          # c^3
                v.scalar_tensor_tensor(u0[:], u0[:], 8.0, rr[:],
                                       ALU.mult, ALU.add)               # 8c^3+r
                v.reciprocal(u2[:], u2[:])
                v.tensor_tensor(cc[:], u0[:], u2[:], ALU.mult)
            sphi = tile4([128, T], "sphi")
            v.tensor_tensor(u0[:], cc[:], cc[:], ALU.mult)
            v.tensor_scalar(u0[:], u0[:], -1.0, 1.0, ALU.mult, ALU.add)  # 1-c^2
            v.tensor_scalar(u0[:], u0[:], 0.0, None, ALU.max)
            s_.activation(sphi[:], u0[:], ACT.Sqrt)

            # lam1 = q + 2p*c ; lam3 = q + 2p*(-c/2 - (sqrt3/2) sphi) ; lam2 = 3q-l1-l3
            lam1 = tile4([128, T], "lam1")
            lam2 = tile4([128, T], "lam2")
            lam3 = tile4([128, T], "lam3")
            v.tensor_tensor(u0[:], pp[:], cc[:], ALU.mult)
            v.tensor_scalar(u0[:], u0[:], 2.0, None, ALU.mult)
            v.tensor_tensor(lam1[:], q[:], u0[:], ALU.add)

            v.tensor_scalar(u0[:], cc[:], -0.5, None, ALU.mult)
            v.scalar_tensor_tensor(u0[:], sphi[:], -math.sqrt(3.0) / 2.0, u0[:],
                                   ALU.mult, ALU.add)
            v.tensor_tensor(u0[:], u0[:], pp[:], ALU.mult)
            v.tensor_scalar(u0[:], u0[:], 2.0, None, ALU.mult)
            v.tensor_tensor(lam3[:], q[:], u0[:], ALU.add)

            v.tensor_scalar(u0[:], q[:], 3.0, None, ALU.mult)
            v.tensor_tensor(lam2[:], u0[:], lam1[:], ALU.subtract)
            v.tensor_tensor(lam2[:], lam2[:], lam3[:], ALU.subtract)

            # s = sqrt(l1) + sqrt(l2) + sign(det)*sqrt(l3)
            for lam in (lam1, lam2, lam3):
                v.tensor_scalar(lam[:], lam[:], 0.0, None, ALU.max)
                s_.activation(lam[:], lam[:], ACT.Sqrt)

            dsign = tile4([128, T], "dsign")
            v.tensor_scalar(dsign[:], det[:], 0.0, None, ALU.is_ge)
            v.tensor_scalar(dsign[:], dsign[:], 2.0, -1.0, ALU.mult, ALU.add)

            ssum = tile4([128, T], "ssum")
            v.tensor_tensor(ssum[:], lam1[:], lam2[:], ALU.add)
            v.tensor_tensor(u0[:], dsign[:], lam3[:], ALU.mult)
            v.tensor_tensor(ssum[:], ssum[:], u0[:], ALU.add)

            # rmsd = sqrt(max((ex + ey - 2 s) / n, 1e-12))
            res = tile4([128, T], "res")
            v.tensor_tensor(res[:], ex[:], ey[:], ALU.add)
            v.scalar_tensor_tensor(res[:], ssum[:], -2.0, res[:], ALU.mult, ALU.add)
            v.tensor_tensor(res[:], res[:], inv_n[:], ALU.mult)
            v.tensor_scalar(res[:], res[:], 1e-12, None, ALU.max)
            s_.activation(res[:], res[:], ACT.Sqrt)

            nc.sync.dma_start(
                out=out[:].rearrange("(t p) -> p t", p=128),
                in_=res[:])

    if split_waits:
        _split_multi_waits(nc)
    return nc


def _split_multi_waits(nc):
    """walrus rejects >1 sync-wait on DVE instruction structs; move extra
    waits onto single-wait NoOp carriers queued just before, same engine."""
    ctr = 0
    for f in nc.m.functions:
        for bb in f.blocks:
            new = []
            for inst in bb.instructions:
                si = inst.sync_info
                if si is not None and si.on_wait and len(si.on_wait) > 1:
                    waits = list(si.on_wait)
                    for w in waits[:-1]:
                        ctr += 1
                        new.append(mybir.InstNoOp(
                            name=f"waitnop-{ctr}", engine=inst.engine,
                            ins=[], outs=[],
                            sync_info=mybir.SyncInfo(on_wait=[w],
                                                     on_update=[])))
                    inst.sync_info = mybir.SyncInfo(on_wait=[waits[-1]],
                                                    on_update=si.on_update)
                new.append(inst)
            bb.instructions = new


_NC_CACHE = None


def _get_nc():
    global _NC_CACHE
    if _NC_CACHE is None:
        _NC_CACHE = _build_kernel()
    return _NC_CACHE


def kernel(input: np.ndarray, target: np.ndarray, num_atoms: np.ndarray,
           **_unused) -> np.ndarray:
    input = np.ascontiguousarray(np.asarray(input, dtype=np.float32))
    target = np.ascontiguousarray(np.asarray(target, dtype=np.float32))
    num_atoms = np.ascontiguousarray(np.asarray(num_atoms, dtype=np.int32))

    nc = _get_nc()
    in_maps = []
    for i in range(N_CORES):
        sl = slice(i * B_CORE, (i + 1) * B_CORE)
        in_maps.append({
            "input": input[sl],
            "target": target[sl],
            "num_atoms": num_atoms[sl],
        })
    res = run_bass_kernel_spmd(nc, in_maps, list(range(N_CORES)))
    outs = [res.results[i]["out"].reshape(B_CORE) for i in range(N_CORES)]
    return np.concatenate(outs).astype(np.float32)


if __name__ == "__main__":
    rng = np.random.default_rng(0)
    inp = rng.standard_normal((B_FULL, ROW), dtype=np.float32)
    tgt = rng.standard_normal((B_FULL, ROW), dtype=np.float32)
    na = rng.integers(8, N_ATOMS + 1, size=(B_FULL,), dtype=np.int32)
    print(kernel(input=inp, target=tgt, num_atoms=na)[:8])



# revision 8
# speedup vs baseline: 1.0101x; 1.0101x over previous
"""Trainium2 Bass kernel for batched masked Kabsch-RMSD (Coords2RMSD loss).

Problem: for each of 4096 samples (1024 max atoms, variable num_atoms),
compute RMSD after optimal rigid alignment (Kabsch). Data-parallel over
8 NeuronCores (512 samples each), samples on SBUF partitions.

Math (per sample, avoids explicit centering):
  mask_i = i < n;  xm = mask*x, ym = mask*y   (interleaved [1024,3] coords)
  Sx_j = sum_i xm_ij, Sy likewise; sxx = sum xm^2, syy = sum ym^2
  R_jk = sum_i xm_ij ym_ik
  Rc = R - Sx Sy^T / n;  ex = sxx - |Sx|^2/n;  ey = syy - |Sy|^2/n
  M = Rc^T Rc;  eigenvalues via Smith's closed form (acos/cos through
  the ScalarE Arctan/Sin tables);  d = sign(det Rc)
  s = sqrt(l1)+sqrt(l2)+d*sqrt(l3);  rmsd = sqrt(max((ex+ey-2s)/n, 1e-12))

Engine split per 128-sample tile:
  DVE : mask compare, 6 fused mask-apply+centroid-sum STTs (strided fp32
        reads, dense de-interleaved writes), 5 fused product+accum STTs
  GPS : 4 fused product+accum STTs (dense reads)
  ACT : 2 Square+accum passes for the norms
Covariance products use scalar_tensor_tensor's accum_out so no separate
reduction passes exist anywhere.
"""

import math
import numpy as np

import concourse.bass as bass
import concourse.mybir as mybir
from concourse.bass_utils import run_bass_kernel_spmd
from concourse.tile import TileContext

F32 = mybir.dt.float32
BF16 = mybir.dt.bfloat16
FP16 = mybir.dt.float16
I32 = mybir.dt.int32
ALU = mybir.AluOpType
ACT = mybir.ActivationFunctionType

N_CORES = 8
B_FULL = 4096
B_CORE = B_FULL // N_CORES        # 512
N_ATOMS = 1024
ROW = 3 * N_ATOMS                 # 3072
N_TILES = B_CORE // 128           # 4

# products assigned to gpsimd (dense reads only; rest go to DVE)
GPS_PRODUCTS = 6
GPS_MASKS = 2
BF16_PRODUCTS = True


def _build_kernel(split_waits: bool = True) -> bass.Bass:
    nc = bass.Bass()

    inp = nc.declare_dram_parameter("input", [B_CORE, ROW], F32, isOutput=False)
    tgt = nc.declare_dram_parameter("target", [B_CORE, ROW], F32, isOutput=False)
    nat = nc.declare_dram_parameter("num_atoms", [B_CORE], I32, isOutput=False)
    out = nc.declare_dram_parameter("out", [B_CORE], F32, isOutput=True)

    with TileContext(nc) as tc:
        with tc.tile_pool(name="p", bufs=1) as pool, \
             tc.tile_pool(name="io", bufs=2) as iop:

            # ---------- one-time setup ----------
            iota_i = pool.tile([128, N_ATOMS], I32, tag="iota_i")
            nc.gpsimd.iota(iota_i[:], pattern=[[1, N_ATOMS]], base=0,
                           channel_multiplier=0)
            # fp16 iota: integers <= 2048 exact; enables DVE fast-mode is_lt
            iota_h = pool.tile([128, N_ATOMS], FP16, tag="iota_h")
            nc.vector.tensor_copy(iota_h[:], iota_i[:])

            # num_atoms -> [128, N_TILES] i32 -> f32, inv_n
            n_i = pool.tile([128, N_TILES], I32, tag="n_i")
            nc.sync.dma_start(out=n_i[:],
                              in_=nat[:].rearrange("(t p) -> p t", p=128))
            nf = pool.tile([128, N_TILES], F32, tag="nf")
            nc.vector.tensor_copy(nf[:], n_i[:])
            inv_n = pool.tile([128, N_TILES], F32, tag="inv_n")
            nc.vector.reciprocal(inv_n[:], nf[:])

            # ---------- per-core stats ----------
            # stats[:, t, 0:9] = R9, [:, t, 9:12] = Sx, [:, t, 12:15] = Sy
            stats = pool.tile([128, N_TILES, 15], F32, tag="stats")
            R9 = stats[:, :, 0:9]
            Sx = stats[:, :, 9:12]
            Sy = stats[:, :, 12:15]
            sxx = pool.tile([128, N_TILES], F32, tag="sxx")
            syy = pool.tile([128, N_TILES], F32, tag="syy")

            act_scr = pool.tile([128, ROW], BF16, tag="act_scr")

            # ---------- main loop over 4 sample tiles ----------
            # Engine split (measured rates): DVE TT bf16 dense hits the 2x
            # fast mode (~0.68us/[128,1024]); STT has no fast mode (~2.4us)
            # so it is banned here. GPS shares DVE's SBUF read port and
            # poisons it (TT 0.68 -> 2.6us when GPS runs) so GPS is idle.
            # ACT (own port) does the strided de-interleaves + squares.
            for t in range(N_TILES):
                x = iop.tile([128, ROW], F32, tag="x")
                y = iop.tile([128, ROW], F32, tag="y")
                nc.sync.dma_start(out=x[:], in_=inp[t * 128 : (t + 1) * 128, :])
                nc.sync.dma_start(out=y[:], in_=tgt[t * 128 : (t + 1) * 128, :])

                # ACT: de-interleave + fp32->bf16 (strided read, dense write)
                xd = iop.tile([128, 3, N_ATOMS], BF16, tag="xd")
                yd = iop.tile([128, 3, N_ATOMS], BF16, tag="yd")
                nc.scalar.activation(xd[:], x[:].rearrange("p (n c) -> p c n", c=3),
                                     ACT.Copy)
                nc.scalar.activation(yd[:], y[:].rearrange("p (n c) -> p c n", c=3),
                                     ACT.Copy)

                # DVE: mask (fast-mode TS), mask-apply (fast-mode TT w/ bcast)
                mask = iop.tile([128, N_ATOMS], BF16, tag="mask")
                nc.vector.tensor_scalar(mask[:], iota_h[:], nf[:, t : t + 1],
                                        None, ALU.is_lt)
                mb = mask[:].rearrange("p n -> p () n").broadcast_to(
                    (128, 3, N_ATOMS))

                # red[:, 0:9] products, [:, 9:12] xm, [:, 12:15] ym
                red = iop.tile([128, 15, N_ATOMS], BF16, tag="red")
                xm = red[:, 9:12, :]
                ym = red[:, 12:15, :]
                nc.vector.tensor_tensor(xm, xd[:], mb, ALU.mult)
                nc.vector.tensor_tensor(ym, yd[:], mb, ALU.mult)

                # 9 covariance products in one fast-mode TT: outer-product
                # broadcast APs (j over xm, k over ym), dense bf16 out
                xmb = red[:, 9:12, :].rearrange("p j n -> p j () n") \
                    .broadcast_to((128, 3, 3, N_ATOMS))
                ymb = red[:, 12:15, :].rearrange("p k n -> p () k n") \
                    .broadcast_to((128, 3, 3, N_ATOMS))
                nc.vector.tensor_tensor(
                    red[:, 0:9, :].rearrange("p (j k) n -> p j k n", k=3),
                    xmb, ymb, ALU.mult)

                # reduce [128, 15, 1024] -> [128, 15] fp32. A direct
                # tensor_reduce runs 1x (fp32 out kills the 2-byte fast
                # mode; measured 16.1us). Binary-fold in fp16 (2x TT) down
                # to 64, then one small 1x reduce (~9.4us total).
                f1 = iop.tile([128, 15, 512], FP16, tag="f1")
                nc.vector.tensor_tensor(f1[:], red[:, :, 0:512],
                                        red[:, :, 512:1024], ALU.add)
                f2 = iop.tile([128, 15, 256], FP16, tag="f2")
                nc.vector.tensor_tensor(f2[:], f1[:, :, 0:256],
                                        f1[:, :, 256:512], ALU.add)
                f3 = iop.tile([128, 15, 128], FP16, tag="f3")
                nc.vector.tensor_tensor(f3[:], f2[:, :, 0:128],
                                        f2[:, :, 128:256], ALU.add)
                f4 = iop.tile([128, 15, 64], FP16, tag="f4")
                nc.vector.tensor_tensor(f4[:], f3[:, :, 0:64],
                                        f3[:, :, 64:128], ALU.add)
                nc.vector.tensor_reduce(stats[:, t, :], f4[:],
                                        mybir.AxisListType.X, ALU.add)

                # norms on ScalarE (Square + accumulate over the whole row)
                nc.scalar.activation(act_scr[:],
                                     red[:, 9:12, :].rearrange("p c n -> p (c n)"),
                                     ACT.Square, accum_out=sxx[:, t : t + 1])
                nc.scalar.activation(act_scr[:],
                                     red[:, 12:15, :].rearrange("p c n -> p (c n)"),
                                     ACT.Square, accum_out=syy[:, t : t + 1])

            # ---------- batched eigensolve / RMSD ([128, N_TILES]) ----------
            T = N_TILES

            def tile4(shape, tag):
                return pool.tile(shape, F32, tag=tag, name=tag)

            v = nc.vector
            s_ = nc.scalar


            # Rc = R9 - (Sx outer Sy) * inv_n
            Rc = tile4([128, T, 9], "Rc")
            t9a = tile4([128, T, 9], "t9a")
            sxb = Sx.broadcast_to((128, T, 3, 3))
            syb = Sy.rearrange("p t k -> p t () k").broadcast_to((128, T, 3, 3))
            v.tensor_tensor(t9a[:].rearrange("p t (j k) -> p t j k", k=3),
                            sxb, syb, ALU.mult)
            invb9 = inv_n[:].rearrange("p t -> p t ()").broadcast_to((128, T, 9))
            v.tensor_tensor(t9a[:], t9a[:], invb9, ALU.mult)
            v.tensor_tensor(Rc[:], R9, t9a[:], ALU.subtract)

            # ex = sxx - |Sx|^2 * inv_n ; ey likewise
            t3 = tile4([128, T, 3], "t3")
            tA = tile4([128, T], "tA")
            tB = tile4([128, T], "tB")
            ex = tile4([128, T], "ex")
            ey = tile4([128, T], "ey")
            v.tensor_tensor(t3[:], Sx, Sx, ALU.mult)
            v.tensor_reduce(tA[:], t3[:], mybir.AxisListType.X, ALU.add)
            v.tensor_tensor(tA[:], tA[:], inv_n[:], ALU.mult)
            v.tensor_tensor(ex[:], sxx[:], tA[:], ALU.subtract)
            v.tensor_tensor(t3[:], Sy, Sy, ALU.mult)
            v.tensor_reduce(tB[:], t3[:], mybir.AxisListType.X, ALU.add)
            v.tensor_tensor(tB[:], tB[:], inv_n[:], ALU.mult)
            v.tensor_tensor(ey[:], syy[:], tB[:], ALU.subtract)

            def col(ap, i):            # [128, T] column i of a [128,T,9] tile
                return ap[:, :, i]

            r00, r01, r02 = (col(Rc, i) for i in range(3))
            r10, r11, r12 = (col(Rc, i) for i in range(3, 6))
            r20, r21, r22 = (col(Rc, i) for i in range(6, 9))

            # det(Rc) via cofactors
            c0 = tile4([128, T], "c0")
            c1 = tile4([128, T], "c1")
            c2 = tile4([128, T], "c2")
            u0 = tile4([128, T], "u0")
            u1 = tile4([128, T], "u1")
            det = tile4([128, T], "det")

            def msub(dst, a, b, c, d):     # dst = a*b - c*d
                v.tensor_tensor(u0[:], a, b, ALU.mult)
                v.tensor_tensor(u1[:], c, d, ALU.mult)
                v.tensor_tensor(dst, u0[:], u1[:], ALU.subtract)

            msub(c0[:], r11, r22, r12, r21)
            msub(c1[:], r10, r22, r12, r20)
            msub(c2[:], r10, r21, r11, r20)
            v.tensor_tensor(c0[:], c0[:], r00, ALU.mult)
            v.tensor_tensor(c1[:], c1[:], r01, ALU.mult)
            v.tensor_tensor(c2[:], c2[:], r02, ALU.mult)
            v.tensor_tensor(det[:], c0[:], c1[:], ALU.subtract)
            v.tensor_tensor(det[:], det[:], c2[:], ALU.add)

            # M = Rc^T Rc (6 unique entries)
            M6 = tile4([128, T, 6], "M6")      # M00 M11 M22 M01 M02 M12
            Rcv = Rc[:].rearrange("p t (j k) -> p t j k", k=3)
            mpairs = [(0, 0), (1, 1), (2, 2), (0, 1), (0, 2), (1, 2)]
            for i, (a, b) in enumerate(mpairs):
                v.tensor_tensor(t3[:], Rcv[:, :, :, a], Rcv[:, :, :, b], ALU.mult)
                v.tensor_reduce(M6[:, :, i], t3[:], mybir.AxisListType.X, ALU.add)

            M00, M11, M22 = (M6[:, :, i] for i in range(3))
            M01, M02, M12 = (M6[:, :, i] for i in range(3, 6))

            # q = tr/3 ; p = sqrt(p2/6) with p2 = sum aii^2 + 2*(off^2)
            q = tile4([128, T], "q")
            v.tensor_tensor(q[:], M00, M11, ALU.add)
            v.tensor_tensor(q[:], q[:], M22, ALU.add)
            v.tensor_scalar(q[:], q[:], 1.0 / 3.0, None, ALU.mult)

            a00 = tile4([128, T], "a00")
            a11 = tile4([128, T], "a11")
            a22 = tile4([128, T], "a22")
            v.tensor_tensor(a00[:], M00, q[:], ALU.subtract)
            v.tensor_tensor(a11[:], M11, q[:], ALU.subtract)
            v.tensor_tensor(a22[:], M22, q[:], ALU.subtract)

            p2 = tile4([128, T], "p2")
            v.tensor_tensor(u0[:], M01, M01, ALU.mult)
            v.tensor_tensor(u1[:], M02, M02, ALU.mult)
            v.tensor_tensor(p2[:], u0[:], u1[:], ALU.add)
            v.tensor_tensor(u0[:], M12, M12, ALU.mult)
            v.tensor_tensor(p2[:], p2[:], u0[:], ALU.add)
            v.tensor_scalar(p2[:], p2[:], 2.0, None, ALU.mult)
            v.tensor_tensor(u0[:], a00[:], a00[:], ALU.mult)
            v.tensor_tensor(p2[:], p2[:], u0[:], ALU.add)
            v.tensor_tensor(u0[:], a11[:], a11[:], ALU.mult)
            v.tensor_tensor(p2[:], p2[:], u0[:], ALU.add)
            v.tensor_tensor(u0[:], a22[:], a22[:], ALU.mult)
            v.tensor_tensor(p2[:], p2[:], u0[:], ALU.add)

            pp = tile4([128, T], "pp")
            v.tensor_scalar(pp[:], p2[:], 1.0 / 6.0, None, ALU.mult)
            s_.activation(pp[:], pp[:], ACT.Sqrt)
            ip = tile4([128, T], "ip")
            v.tensor_scalar(u0[:], pp[:], 1e-20, None, ALU.max)
            v.reciprocal(ip[:], u0[:])

            # detB = det(M - qI) ; r = detB * ip^3 / 2, clamped to [-1, 1]
            detB = tile4([128, T], "detB")
            msub(c0[:], a11[:], a22[:], M12, M12)
            msub(c1[:], M01, a22[:], M12, M02)
            msub(c2[:], M01, M12, a11[:], M02)
            v.tensor_tensor(c0[:], c0[:], a00[:], ALU.mult)
            v.tensor_tensor(c1[:], c1[:], M01, ALU.mult)
            v.tensor_tensor(c2[:], c2[:], M02, ALU.mult)
            v.tensor_tensor(detB[:], c0[:], c1[:], ALU.subtract)
            v.tensor_tensor(detB[:], detB[:], c2[:], ALU.add)

            rr = tile4([128, T], "rr")
            v.tensor_tensor(u0[:], ip[:], ip[:], ALU.mult)
            v.tensor_tensor(u0[:], u0[:], ip[:], ALU.mult)
            v.tensor_tensor(rr[:], detB[:], u0[:], ALU.mult)
            v.tensor_scalar(rr[:], rr[:], 0.5, None, ALU.mult)
            v.tensor_scalar(rr[:], rr[:], 1.0, -1.0, ALU.min, ALU.max)

            # c = cos(acos(r)/3) is the root of 4c^3-3c=r in [1/2,1].
            # Newton (table-free, avoids ACT Arctan/Sin set loads):
            #   c <- (8c^3 + r) / (12c^2 - 3), from c=1, 7 iterations.
            # Errors at the degenerate r=-1 corner are benign: lam1+lam2 is
            # trace-compensated and d(t3)/dc = 0 at c=1/2.
            cc = tile4([128, T], "cc")
            u2 = tile4([128, T], "u2")
            # init: quadratic fit of cos(acos(r)/3), max err ~5e-3
            v.tensor_scalar(cc[:], rr[:], -0.116, 0.25, ALU.mult, ALU.add)
            v.scalar_tensor_tensor(cc[:], rr[:], 1.0, cc[:], ALU.mult, ALU.mult)
            v.tensor_scalar(cc[:], cc[:], 1.0, 0.866, ALU.mult, ALU.add)
            for _ in range(3):
                v.tensor_tensor(u0[:], cc[:], cc[:], ALU.mult)          # c^2
                v.tensor_scalar(u2[:], u0[:], 12.0, -3.0, ALU.mult, ALU.add)
                v.tensor_tensor(u0[:], u0[:], cc[:], ALU.mult)          # c^3
                v.scalar_tensor_tensor(u0[:], u0[:], 8.0, rr[:],
                                       ALU.mult, ALU.add)               # 8c^3+r
                v.reciprocal(u2[:], u2[:])
                v.tensor_tensor(cc[:], u0[:], u2[:], ALU.mult)
            sphi = tile4([128, T], "sphi")
            v.tensor_tensor(u0[:], cc[:], cc[:], ALU.mult)
            v.tensor_scalar(u0[:], u0[:], -1.0, 1.0, ALU.mult, ALU.add)  # 1-c^2
            v.tensor_scalar(u0[:], u0[:], 0.0, None, ALU.max)
            s_.activation(sphi[:], u0[:], ACT.Sqrt)

            # lam1 = q + 2p*c ; lam3 = q + 2p*(-c/2 - (sqrt3/2) sphi) ; lam2 = 3q-l1-l3
            lam1 = tile4([128, T], "lam1")
            lam2 = tile4([128, T], "lam2")
            lam3 = tile4([128, T], "lam3")
            v.tensor_tensor(u0[:], pp[:], cc[:], ALU.mult)
            v.tensor_scalar(u0[:], u0[:], 2.0, None, ALU.mult)
            v.tensor_tensor(lam1[:], q[:], u0[:], ALU.add)

            v.tensor_scalar(u0[:], cc[:], -0.5, None, ALU.mult)
            v.scalar_tensor_tensor(u0[:], sphi[:], -math.sqrt(3.0) / 2.0, u0[:],
                                   ALU.mult, ALU.add)
            v.tensor_tensor(u0[:], u0[:], pp[:], ALU.mult)
            v.tensor_scalar(u0[:], u0[:], 2.0, None, ALU.mult)
            v.tensor_tensor(lam3[:], q[:], u0[:], ALU.add)

            v.tensor_scalar(u0[:], q[:], 3.0, None, ALU.mult)
            v.tensor_tensor(lam2[:], u0[:], lam1[:], ALU.subtract)
            v.tensor_tensor(lam2[:], lam2[:], lam3[:], ALU.subtract)

            # s = sqrt(l1) + sqrt(l2) + sign(det)*sqrt(l3)
            for lam in (lam1, lam2, lam3):
                v.tensor_scalar(lam[:], lam[:], 0.0, None, ALU.max)
                s_.activation(lam[:], lam[:], ACT.Sqrt)

            dsign = tile4([128, T], "dsign")
            v.tensor_scalar(dsign[:], det[:], 0.0, None, ALU.is_ge)
            v.tensor_scalar(dsign[:], dsign[:], 2.0, -1.0, ALU.mult, ALU.add)

            ssum = tile4([128, T], "ssum")
            v.tensor_tensor(ssum[:], lam1[:], lam2[:], ALU.add)
            v.tensor_tensor(u0[:], dsign[:], lam3[:], ALU.mult)
            v.tensor_tensor(ssum[:], ssum[:], u0[:], ALU.add)

            # rmsd = sqrt(max((ex + ey - 2 s) / n, 1e-12))
            res = tile4([128, T], "res")
            v.tensor_tensor(res[:], ex[:], ey[:], ALU.add)
            v.scalar_tensor_tensor(res[:], ssum[:], -2.0, res[:], ALU.mult, ALU.add)
            v.tensor_tensor(res[:], res[:], inv_n[:], ALU.mult)
            v.tensor_scalar(res[:], res[:], 1e-12, None, ALU.max)
            s_.activation(res[:], res[:], ACT.Sqrt)

            nc.sync.dma_start(
                out=out[:].rearrange("(t p) -> p t", p=128),
                in_=res[:])

    if split_waits:
        _split_multi_waits(nc)
    return nc


def _split_multi_waits(nc):
    """walrus rejects >1 sync-wait on DVE instruction structs; move extra
    waits onto single-wait NoOp carriers queued just before, same engine."""
    ctr = 0
    for f in nc.m.functions:
        for bb in f.blocks:
            new = []
            for inst in bb.instructions:
                si = inst.sync_info
                if si is not None and si.on_wait and len(si.on_wait) > 1:
                    waits = list(si.on_wait)
                    for w in waits[:-1]:
                        ctr += 1
                        new.append(mybir.InstNoOp(
                            name=f"waitnop-{ctr}", engine=inst.engine,
                            ins=[], outs=[],
                            sync_info=mybir.SyncInfo(on_wait=[w],
                                                     on_update=[])))
                    inst.sync_info = mybir.SyncInfo(on_wait=[waits[-1]],
                                                    on_update=si.on_update)
                new.append(inst)
            bb.instructions = new


_NC_CACHE = None


def _get_nc():
    global _NC_CACHE
    if _NC_CACHE is None:
        _NC_CACHE = _build_kernel()
    return _NC_CACHE


def kernel(input: np.ndarray, target: np.ndarray, num_atoms: np.ndarray,
           **_unused) -> np.ndarray:
    input = np.ascontiguousarray(np.asarray(input, dtype=np.float32))
    target = np.ascontiguousarray(np.asarray(target, dtype=np.float32))
    num_atoms = np.ascontiguousarray(np.asarray(num_atoms, dtype=np.int32))

    nc = _get_nc()
    in_maps = []
    for i in range(N_CORES):
        sl = slice(i * B_CORE, (i + 1) * B_CORE)
        in_maps.append({
            "input": input[sl],
            "target": target[sl],
            "num_atoms": num_atoms[sl],
        })
    res = run_bass_kernel_spmd(nc, in_maps, list(range(N_CORES)))
    outs = [res.results[i]["out"].reshape(B_CORE) for i in range(N_CORES)]
    return np.concatenate(outs).astype(np.float32)


if __name__ == "__main__":
    rng = np.random.default_rng(0)
    inp = rng.standard_normal((B_FULL, ROW), dtype=np.float32)
    tgt = rng.standard_normal((B_FULL, ROW), dtype=np.float32)
    na = rng.integers(8, N_ATOMS + 1, size=(B_FULL,), dtype=np.int32)
    print(kernel(input=inp, target=tgt, num_atoms=na)[:8])



# revision 9
# speedup vs baseline: 1.0559x; 1.0454x over previous
"""Trainium2 Bass kernel for batched masked Kabsch-RMSD (Coords2RMSD loss).

Problem: for each of 4096 samples (1024 max atoms, variable num_atoms),
compute RMSD after optimal rigid alignment (Kabsch). Data-parallel over
8 NeuronCores (512 samples each), samples on SBUF partitions.

Math (per sample, avoids explicit centering):
  mask_i = i < n;  xm = mask*x, ym = mask*y   (interleaved [1024,3] coords)
  Sx_j = sum_i xm_ij, Sy likewise; sxx = sum xm^2, syy = sum ym^2
  R_jk = sum_i xm_ij ym_ik
  Rc = R - Sx Sy^T / n;  ex = sxx - |Sx|^2/n;  ey = syy - |Sy|^2/n
  M = Rc^T Rc;  eigenvalues via Smith's closed form (acos/cos through
  the ScalarE Arctan/Sin tables);  d = sign(det Rc)
  s = sqrt(l1)+sqrt(l2)+d*sqrt(l3);  rmsd = sqrt(max((ex+ey-2s)/n, 1e-12))

Engine split per 128-sample tile:
  DVE : mask compare, 6 fused mask-apply+centroid-sum STTs (strided fp32
        reads, dense de-interleaved writes), 5 fused product+accum STTs
  GPS : 4 fused product+accum STTs (dense reads)
  ACT : 2 Square+accum passes for the norms
Covariance products use scalar_tensor_tensor's accum_out so no separate
reduction passes exist anywhere.
"""

import math
import numpy as np

import concourse.bass as bass
import concourse.mybir as mybir
from concourse.bass_utils import run_bass_kernel_spmd
from concourse.tile import TileContext

F32 = mybir.dt.float32
BF16 = mybir.dt.bfloat16
FP16 = mybir.dt.float16
I32 = mybir.dt.int32
ALU = mybir.AluOpType
ACT = mybir.ActivationFunctionType

N_CORES = 8
B_FULL = 4096
B_CORE = B_FULL // N_CORES        # 512
N_ATOMS = 1024
ROW = 3 * N_ATOMS                 # 3072
N_TILES = B_CORE // 128           # 4

# products assigned to gpsimd (dense reads only; rest go to DVE)
GPS_PRODUCTS = 6
GPS_MASKS = 2
BF16_PRODUCTS = True


def _build_kernel(split_waits: bool = True) -> bass.Bass:
    nc = bass.Bass()

    inp = nc.declare_dram_parameter("input", [B_CORE, ROW], F32, isOutput=False)
    tgt = nc.declare_dram_parameter("target", [B_CORE, ROW], F32, isOutput=False)
    nat = nc.declare_dram_parameter("num_atoms", [B_CORE], I32, isOutput=False)
    out = nc.declare_dram_parameter("out", [B_CORE], F32, isOutput=True)

    with TileContext(nc) as tc:
        with tc.tile_pool(name="p", bufs=1) as pool, \
             tc.tile_pool(name="io", bufs=2) as iop:

            # ---------- one-time setup ----------
            iota_i = pool.tile([128, N_ATOMS], I32, tag="iota_i")
            nc.gpsimd.iota(iota_i[:], pattern=[[1, N_ATOMS]], base=0,
                           channel_multiplier=0)
            # fp16 iota: integers <= 2048 exact; enables DVE fast-mode is_lt
            iota_h = pool.tile([128, N_ATOMS], FP16, tag="iota_h")
            nc.vector.tensor_copy(iota_h[:], iota_i[:])

            # num_atoms -> [128, N_TILES] i32 -> f32, inv_n
            n_i = pool.tile([128, N_TILES], I32, tag="n_i")
            nc.sync.dma_start(out=n_i[:],
                              in_=nat[:].rearrange("(t p) -> p t", p=128))
            nf = pool.tile([128, N_TILES], F32, tag="nf")
            nc.vector.tensor_copy(nf[:], n_i[:])
            inv_n = pool.tile([128, N_TILES], F32, tag="inv_n")
            nc.vector.reciprocal(inv_n[:], nf[:])

            # ---------- per-core stats ----------
            # stats[:, t, 0:9] = R9, [:, t, 9:12] = Sy; Sx via ACT accums
            stats = pool.tile([128, N_TILES, 12], F32, tag="stats")
            R9 = stats[:, :, 0:9]
            Sy = stats[:, :, 9:12]
            Sxt = pool.tile([128, N_TILES, 3], F32, tag="Sxt")
            Sx = Sxt[:]
            sxx = pool.tile([128, N_TILES], F32, tag="sxx")
            syy = pool.tile([128, N_TILES], F32, tag="syy")

            act_scr = pool.tile([128, ROW], BF16, tag="act_scr")

            # ---------- main loop over 4 sample tiles ----------
            # Engine split (measured rates): DVE TT bf16 dense hits the 2x
            # fast mode (~0.68us/[128,1024]); STT has no fast mode (~2.4us)
            # so it is banned here. GPS shares DVE's SBUF read port and
            # poisons it (TT 0.68 -> 2.6us when GPS runs) so GPS is idle.
            # ACT (own port) does the strided de-interleaves + squares.
            for t in range(N_TILES):
                x = iop.tile([128, ROW], F32, tag="x")
                y = iop.tile([128, ROW], F32, tag="y")
                nc.sync.dma_start(out=x[:], in_=inp[t * 128 : (t + 1) * 128, :])
                nc.sync.dma_start(out=y[:], in_=tgt[t * 128 : (t + 1) * 128, :])

                # ACT: de-interleave + fp32->bf16 (strided read, dense write)
                xd = iop.tile([128, 3, N_ATOMS], BF16, tag="xd")
                yd = iop.tile([128, 3, N_ATOMS], BF16, tag="yd")
                nc.scalar.activation(xd[:], x[:].rearrange("p (n c) -> p c n", c=3),
                                     ACT.Copy)
                nc.scalar.activation(yd[:], y[:].rearrange("p (n c) -> p c n", c=3),
                                     ACT.Copy)

                # DVE: mask (fast-mode TS), mask-apply (fast-mode TT w/ bcast)
                mask = iop.tile([128, N_ATOMS], BF16, tag="mask")
                nc.vector.tensor_scalar(mask[:], iota_h[:], nf[:, t : t + 1],
                                        None, ALU.is_lt)
                mb = mask[:].rearrange("p n -> p () n").broadcast_to(
                    (128, 3, N_ATOMS))

                # red[:, 0:9] products, [:, 9:12] ym, [:, 12:15] xm
                red = iop.tile([128, 15, N_ATOMS], BF16, tag="red")
                xm = red[:, 12:15, :]
                ym = red[:, 9:12, :]
                nc.vector.tensor_tensor(xm, xd[:], mb, ALU.mult)
                nc.vector.tensor_tensor(ym, yd[:], mb, ALU.mult)

                # 9 covariance products in one fast-mode TT: outer-product
                # broadcast APs (j over xm, k over ym), dense bf16 out
                xmb = red[:, 12:15, :].rearrange("p j n -> p j () n") \
                    .broadcast_to((128, 3, 3, N_ATOMS))
                ymb = red[:, 9:12, :].rearrange("p k n -> p () k n") \
                    .broadcast_to((128, 3, 3, N_ATOMS))
                nc.vector.tensor_tensor(
                    red[:, 0:9, :].rearrange("p (j k) n -> p j k n", k=3),
                    xmb, ymb, ALU.mult)

                # reduce [128, 15, 1024] -> [128, 15] fp32. A direct
                # tensor_reduce runs 1x (fp32 out kills the 2-byte fast
                # mode; measured 16.1us). Binary-fold in fp16 (2x TT) down
                # to 64, then one small 1x reduce (~9.4us total).
                f1 = iop.tile([128, 12, 512], FP16, tag="f1")
                nc.vector.tensor_tensor(f1[:], red[:, 0:12, 0:512],
                                        red[:, 0:12, 512:1024], ALU.add)
                f2 = iop.tile([128, 12, 256], FP16, tag="f2")
                nc.vector.tensor_tensor(f2[:], f1[:, :, 0:256],
                                        f1[:, :, 256:512], ALU.add)
                f3 = iop.tile([128, 12, 128], FP16, tag="f3")
                nc.vector.tensor_tensor(f3[:], f2[:, :, 0:128],
                                        f2[:, :, 128:256], ALU.add)
                f4 = iop.tile([128, 12, 64], FP16, tag="f4")
                nc.vector.tensor_tensor(f4[:], f3[:, :, 0:64],
                                        f3[:, :, 64:128], ALU.add)
                nc.vector.tensor_reduce(stats[:, t, :], f4[:],
                                        mybir.AxisListType.X, ALU.add)
                # Sx on ACT (Copy+accum per coord), off the DVE fold tree
                for j in range(3):
                    nc.scalar.activation(act_scr[:, 0:N_ATOMS],
                                         red[:, 12 + j, :], ACT.Copy,
                                         accum_out=Sxt[:, t, j : j + 1])

                # norms on ScalarE (Square + accumulate over the whole row)
                nc.scalar.activation(act_scr[:],
                                     red[:, 12:15, :].rearrange("p c n -> p (c n)"),
                                     ACT.Square, accum_out=sxx[:, t : t + 1])
                nc.scalar.activation(act_scr[:],
                                     red[:, 9:12, :].rearrange("p c n -> p (c n)"),
                                     ACT.Square, accum_out=syy[:, t : t + 1])

            # ---------- batched eigensolve / RMSD ([128, N_TILES]) ----------
            T = N_TILES

            def tile4(shape, tag):
                return pool.tile(shape, F32, tag=tag, name=tag)

            v = nc.vector
            s_ = nc.scalar


            # Rc = R9 - (Sx outer Sy) * inv_n
            Rc = tile4([128, T, 9], "Rc")
            t9a = tile4([128, T, 9], "t9a")
            sxb = Sx.rearrange("p t j -> p t j ()").broadcast_to((128, T, 3, 3))
            syb = Sy.rearrange("p t k -> p t () k").broadcast_to((128, T, 3, 3))
            v.tensor_tensor(t9a[:].rearrange("p t (j k) -> p t j k", k=3),
                            sxb, syb, ALU.mult)
            invb9 = inv_n[:].rearrange("p t -> p t ()").broadcast_to((128, T, 9))
            v.tensor_tensor(t9a[:], t9a[:], invb9, ALU.mult)
            v.tensor_tensor(Rc[:], R9, t9a[:], ALU.subtract)

            # ex = sxx - |Sx|^2 * inv_n ; ey likewise
            t3 = tile4([128, T, 3], "t3")
            tA = tile4([128, T], "tA")
            tB = tile4([128, T], "tB")
            ex = tile4([128, T], "ex")
            ey = tile4([128, T], "ey")
            v.tensor_tensor(t3[:], Sx, Sx, ALU.mult)
            v.tensor_reduce(tA[:], t3[:], mybir.AxisListType.X, ALU.add)
            v.tensor_tensor(tA[:], tA[:], inv_n[:], ALU.mult)
            v.tensor_tensor(ex[:], sxx[:], tA[:], ALU.subtract)
            v.tensor_tensor(t3[:], Sy, Sy, ALU.mult)
            v.tensor_reduce(tB[:], t3[:], mybir.AxisListType.X, ALU.add)
            v.tensor_tensor(tB[:], tB[:], inv_n[:], ALU.mult)
            v.tensor_tensor(ey[:], syy[:], tB[:], ALU.subtract)

            def col(ap, i):            # [128, T] column i of a [128,T,9] tile
                return ap[:, :, i]

            r00, r01, r02 = (col(Rc, i) for i in range(3))
            r10, r11, r12 = (col(Rc, i) for i in range(3, 6))
            r20, r21, r22 = (col(Rc, i) for i in range(6, 9))

            # det(Rc) via cofactors
            c0 = tile4([128, T], "c0")
            c1 = tile4([128, T], "c1")
            c2 = tile4([128, T], "c2")
            u0 = tile4([128, T], "u0")
            u1 = tile4([128, T], "u1")
            det = tile4([128, T], "det")

            def msub(dst, a, b, c, d):     # dst = a*b - c*d
                v.tensor_tensor(u0[:], a, b, ALU.mult)
                v.tensor_tensor(u1[:], c, d, ALU.mult)
                v.tensor_tensor(dst, u0[:], u1[:], ALU.subtract)

            msub(c0[:], r11, r22, r12, r21)
            msub(c1[:], r10, r22, r12, r20)
            msub(c2[:], r10, r21, r11, r20)
            v.tensor_tensor(c0[:], c0[:], r00, ALU.mult)
            v.tensor_tensor(c1[:], c1[:], r01, ALU.mult)
            v.tensor_tensor(c2[:], c2[:], r02, ALU.mult)
            v.tensor_tensor(det[:], c0[:], c1[:], ALU.subtract)
            v.tensor_tensor(det[:], det[:], c2[:], ALU.add)

            # M = Rc^T Rc (6 unique entries)
            M6 = tile4([128, T, 6], "M6")      # M00 M11 M22 M01 M02 M12
            Rcv = Rc[:].rearrange("p t (j k) -> p t j k", k=3)
            mpairs = [(0, 0), (1, 1), (2, 2), (0, 1), (0, 2), (1, 2)]
            for i, (a, b) in enumerate(mpairs):
                v.tensor_tensor(t3[:], Rcv[:, :, :, a], Rcv[:, :, :, b], ALU.mult)
                v.tensor_reduce(M6[:, :, i], t3[:], mybir.AxisListType.X, ALU.add)

            M00, M11, M22 = (M6[:, :, i] for i in range(3))
            M01, M02, M12 = (M6[:, :, i] for i in range(3, 6))

            # q = tr/3 ; p = sqrt(p2/6) with p2 = sum aii^2 + 2*(off^2)
            q = tile4([128, T], "q")
            v.tensor_tensor(q[:], M00, M11, ALU.add)
            v.tensor_tensor(q[:], q[:], M22, ALU.add)
            v.tensor_scalar(q[:], q[:], 1.0 / 3.0, None, ALU.mult)

            a00 = tile4([128, T], "a00")
            a11 = tile4([128, T], "a11")
            a22 = tile4([128, T], "a22")
            v.tensor_tensor(a00[:], M00, q[:], ALU.subtract)
            v.tensor_tensor(a11[:], M11, q[:], ALU.subtract)
            v.tensor_tensor(a22[:], M22, q[:], ALU.subtract)

            p2 = tile4([128, T], "p2")
            v.tensor_tensor(u0[:], M01, M01, ALU.mult)
            v.tensor_tensor(u1[:], M02, M02, ALU.mult)
            v.tensor_tensor(p2[:], u0[:], u1[:], ALU.add)
            v.tensor_tensor(u0[:], M12, M12, ALU.mult)
            v.tensor_tensor(p2[:], p2[:], u0[:], ALU.add)
            v.tensor_scalar(p2[:], p2[:], 2.0, None, ALU.mult)
            v.tensor_tensor(u0[:], a00[:], a00[:], ALU.mult)
            v.tensor_tensor(p2[:], p2[:], u0[:], ALU.add)
            v.tensor_tensor(u0[:], a11[:], a11[:], ALU.mult)
            v.tensor_tensor(p2[:], p2[:], u0[:], ALU.add)
            v.tensor_tensor(u0[:], a22[:], a22[:], ALU.mult)
            v.tensor_tensor(p2[:], p2[:], u0[:], ALU.add)

            pp = tile4([128, T], "pp")
            v.tensor_scalar(pp[:], p2[:], 1.0 / 6.0, None, ALU.mult)
            s_.activation(pp[:], pp[:], ACT.Sqrt)
            ip = tile4([128, T], "ip")
            v.tensor_scalar(u0[:], pp[:], 1e-20, None, ALU.max)
            v.reciprocal(ip[:], u0[:])

            # detB = det(M - qI) ; r = detB * ip^3 / 2, clamped to [-1, 1]
            detB = tile4([128, T], "detB")
            msub(c0[:], a11[:], a22[:], M12, M12)
            msub(c1[:], M01, a22[:], M12, M02)
            msub(c2[:], M01, M12, a11[:], M02)
            v.tensor_tensor(c0[:], c0[:], a00[:], ALU.mult)
            v.tensor_tensor(c1[:], c1[:], M01, ALU.mult)
            v.tensor_tensor(c2[:], c2[:], M02, ALU.mult)
            v.tensor_tensor(detB[:], c0[:], c1[:], ALU.subtract)
            v.tensor_tensor(detB[:], detB[:], c2[:], ALU.add)

            rr = tile4([128, T], "rr")
            v.tensor_tensor(u0[:], ip[:], ip[:], ALU.mult)
            v.tensor_tensor(u0[:], u0[:], ip[:], ALU.mult)
            v.tensor_tensor(rr[:], detB[:], u0[:], ALU.mult)
            v.tensor_scalar(rr[:], rr[:], 0.5, None, ALU.mult)
            v.tensor_scalar(rr[:], rr[:], 1.0, -1.0, ALU.min, ALU.max)

            # c = cos(acos(r)/3) is the root of 4c^3-3c=r in [1/2,1].
            # Newton (table-free, avoids ACT Arctan/Sin set loads):
            #   c <- (8c^3 + r) / (12c^2 - 3), from c=1, 7 iterations.
            # Errors at the degenerate r=-1 corner are benign: lam1+lam2 is
            # trace-compensated and d(t3)/dc = 0 at c=1/2.
            cc = tile4([128, T], "cc")
            u2 = tile4([128, T], "u2")
            # init: quadratic fit of cos(acos(r)/3), max err ~5e-3
            v.tensor_scalar(cc[:], rr[:], -0.116, 0.25, ALU.mult, ALU.add)
            v.scalar_tensor_tensor(cc[:], rr[:], 1.0, cc[:], ALU.mult, ALU.mult)
            v.tensor_scalar(cc[:], cc[:], 1.0, 0.866, ALU.mult, ALU.add)
            for _ in range(3):
                v.tensor_tensor(u0[:], cc[:], cc[:], ALU.mult)          # c^2
                v.tensor_scalar(u2[:], u0[:], 12.0, -3.0, ALU.mult, ALU.add)
                v.tensor_tensor(u0[:], u0[:], cc[:], ALU.mult)          # c^3
                v.scalar_tensor_tensor(u0[:], u0[:], 8.0, rr[:],
                                       ALU.mult, ALU.add)               # 8c^3+r
                v.reciprocal(u2[:], u2[:])
                v.tensor_tensor(cc[:], u0[:], u2[:], ALU.mult)
            sphi = tile4([128, T], "sphi")
            v.tensor_tensor(u0[:], cc[:], cc[:], ALU.mult)
            v.tensor_scalar(u0[:], u0[:], -1.0, 1.0, ALU.mult, ALU.add)  # 1-c^2
            v.tensor_scalar(u0[:], u0[:], 0.0, None, ALU.max)
            s_.activation(sphi[:], u0[:], ACT.Sqrt)

            # lam1 = q + 2p*c ; lam3 = q + 2p*(-c/2 - (sqrt3/2) sphi) ; lam2 = 3q-l1-l3
            lam1 = tile4([128, T], "lam1")
            lam2 = tile4([128, T], "lam2")
            lam3 = tile4([128, T], "lam3")
            v.tensor_tensor(u0[:], pp[:], cc[:], ALU.mult)
            v.tensor_scalar(u0[:], u0[:], 2.0, None, ALU.mult)
            v.tensor_tensor(lam1[:], q[:], u0[:], ALU.add)

            v.tensor_scalar(u0[:], cc[:], -0.5, None, ALU.mult)
            v.scalar_tensor_tensor(u0[:], sphi[:], -math.sqrt(3.0) / 2.0, u0[:],
                                   ALU.mult, ALU.add)
            v.tensor_tensor(u0[:], u0[:], pp[:], ALU.mult)
            v.tensor_scalar(u0[:], u0[:], 2.0, None, ALU.mult)
            v.tensor_tensor(lam3[:], q[:], u0[:], ALU.add)

            v.tensor_scalar(u0[:], q[:], 3.0, None, ALU.mult)
            v.tensor_tensor(lam2[:], u0[:], lam1[:], ALU.subtract)
            v.tensor_tensor(lam2[:], lam2[:], lam3[:], ALU.subtract)

            # s = sqrt(l1) + sqrt(l2) + sign(det)*sqrt(l3)
            for lam in (lam1, lam2, lam3):
                v.tensor_scalar(lam[:], lam[:], 0.0, None, ALU.max)
                s_.activation(lam[:], lam[:], ACT.Sqrt)

            dsign = tile4([128, T], "dsign")
            v.tensor_scalar(dsign[:], det[:], 0.0, None, ALU.is_ge)
            v.tensor_scalar(dsign[:], dsign[:], 2.0, -1.0, ALU.mult, ALU.add)

            ssum = tile4([128, T], "ssum")
            v.tensor_tensor(ssum[:], lam1[:], lam2[:], ALU.add)
            v.tensor_tensor(u0[:], dsign[:], lam3[:], ALU.mult)
            v.tensor_tensor(ssum[:], ssum[:], u0[:], ALU.add)

            # rmsd = sqrt(max((ex + ey - 2 s) / n, 1e-12))
            res = tile4([128, T], "res")
            v.tensor_tensor(res[:], ex[:], ey[:], ALU.add)
            v.scalar_tensor_tensor(res[:], ssum[:], -2.0, res[:], ALU.mult, ALU.add)
            v.tensor_tensor(res[:], res[:], inv_n[:], ALU.mult)
            v.tensor_scalar(res[:], res[:], 1e-12, None, ALU.max)
            s_.activation(res[:], res[:], ACT.Sqrt)

            nc.sync.dma_start(
                out=out[:].rearrange("(t p) -> p t", p=128),
                in_=res[:])

    if split_waits:
        _split_multi_waits(nc)
    return nc


def _split_multi_waits(nc):
    """walrus rejects >1 sync-wait on DVE instruction structs; move extra
    waits onto single-wait NoOp carriers queued just before, same engine."""
    ctr = 0
    for f in nc.m.functions:
        for bb in f.blocks:
            new = []
            for inst in bb.instructions:
                si = inst.sync_info
                if si is not None and si.on_wait and len(si.on_wait) > 1:
                    waits = list(si.on_wait)
                    for w in waits[:-1]:
                        ctr += 1
                        new.append(mybir.InstNoOp(
                            name=f"waitnop-{ctr}", engine=inst.engine,
                            ins=[], outs=[],
                            sync_info=mybir.SyncInfo(on_wait=[w],
                                                     on_update=[])))
                    inst.sync_info = mybir.SyncInfo(on_wait=[waits[-1]],
                                                    on_update=si.on_update)
                new.append(inst)
            bb.instructions = new


_NC_CACHE = None


def _get_nc():
    global _NC_CACHE
    if _NC_CACHE is None:
        _NC_CACHE = _build_kernel()
    return _NC_CACHE


def kernel(input: np.ndarray, target: np.ndarray, num_atoms: np.ndarray,
           **_unused) -> np.ndarray:
    input = np.ascontiguousarray(np.asarray(input, dtype=np.float32))
    target = np.ascontiguousarray(np.asarray(target, dtype=np.float32))
    num_atoms = np.ascontiguousarray(np.asarray(num_atoms, dtype=np.int32))

    nc = _get_nc()
    in_maps = []
    for i in range(N_CORES):
        sl = slice(i * B_CORE, (i + 1) * B_CORE)
        in_maps.append({
            "input": input[sl],
            "target": target[sl],
            "num_atoms": num_atoms[sl],
        })
    res = run_bass_kernel_spmd(nc, in_maps, list(range(N_CORES)))
    outs = [res.results[i]["out"].reshape(B_CORE) for i in range(N_CORES)]
    return np.concatenate(outs).astype(np.float32)


if __name__ == "__main__":
    rng = np.random.default_rng(0)
    inp = rng.standard_normal((B_FULL, ROW), dtype=np.float32)
    tgt = rng.standard_normal((B_FULL, ROW), dtype=np.float32)
    na = rng.integers(8, N_ATOMS + 1, size=(B_FULL,), dtype=np.int32)
    print(kernel(input=inp, target=tgt, num_atoms=na)[:8])

